# revision 38
# baseline (speedup 1.0000x reference)
"""Trainium2 Bass kernel for nn_DecoderBlock (linear-attention decoder block).

Sharding: token-parallel across 8 cores (each core owns (B*T)/8 = 256 rows of
the flattened [B*T, C] token stream; weights replicated per core). Linear
attention is exact: intra-chunk causal block plus cross-core KV prefix states
carried by two small AllGathers (SA states early, CA states later) so the
collectives overlap local compute. Activations stay transposed
([C partitions, tokens free]) so every GEMM lhsT is a plain DRAM weight slice.

Performance notes (vs the first working version):
 - whole datapath in fp16 (residuals included); inputs/outputs host-cast.
 - x/memory loaded with DMA-transpose (XBAR) instead of PE transposes.
 - all per-head-pair elementwise work batched into [128, 8*R] tiles
   (engine op cost scales with free-dim length, partitions are free).
 - denominators: 16 single-matmul chains into disjoint partition rows of one
   PSUM bank, one scalar-engine table reciprocal, 16 cheap gpsimd partition
   broadcasts (replaces 64 tiny matmuls + full-tile vector reciprocals).
 - PSUM statically partitioned: 4 banks GEMM, 2 banks attention, 2 scratch.
"""

import math
import numpy as np
from dataclasses import dataclass

P = 128
HD = 64  # head dim (fixed: C // n_head)
HH = HD // 2
LN_EPS = 1e-5
DSCALE = 4096.0  # denominator pre-scale: keeps 1/denom in fp16 normal range


@dataclass(frozen=True)
class Cfg:
    B: int = 2
    T: int = 1024
    C: int = 1024
    H: int = 16
    NCORE: int = 8
    mm: str = "fp16"  # GEMM dtype: fp16 | bf16
    gelu: str = "table"
    debug_dump: bool = False

    @property
    def R(self):
        return self.B * self.T // self.NCORE

    @property
    def KC(self):
        return self.C // P

    @property
    def NT(self):
        return math.ceil(self.R / P)

    @property
    def NPAIR(self):
        return self.H // 2

    @property
    def FR(self):
        return self.NPAIR * self.R

    @property
    def AGW(self):
        return HD * self.NPAIR + self.NPAIR  # 520: states + kf sums


# ---------------------------------------------------------------------------
# Host-side helpers
# ---------------------------------------------------------------------------

def _rope_tables(T):
    inv = 1.0 / (10000.0 ** (np.arange(0, HD, 2, dtype=np.float64) / HD))
    freqs = np.outer(np.arange(T), inv)
    emb = np.concatenate([freqs, freqs], axis=-1)
    return np.cos(emb).astype(np.float32), np.sin(emb).astype(np.float32)


def _pack_cols(vecs):
    flat = np.concatenate([np.asarray(v, np.float32).ravel() for v in vecs])
    assert flat.size % P == 0
    return np.ascontiguousarray(flat.reshape(-1, P).T)


def _np_wdt(mm):
    if mm == "fp16":
        return np.float16
    import ml_dtypes
    return ml_dtypes.bfloat16


def _host_inputs(cfg: Cfg, inputs):
    B, T, C, NC = cfg.B, cfg.T, cfg.C, cfg.NCORE
    R, NPAIR = cfg.R, cfg.NPAIR
    wdt = _np_wdt(cfg.mm)
    xf = np.ascontiguousarray(
        np.asarray(inputs["x"], np.float32).reshape(B * T, C).astype(wdt))
    mf = np.ascontiguousarray(
        np.asarray(inputs["memory"], np.float32).reshape(B * T, C).astype(wdt))
    cos, sin = _rope_tables(T)

    params = _pack_cols([inputs[k] for k in (
        "ln1_g", "ln1_b", "ln2_g", "ln2_b", "ln3_g", "ln3_b",
        "sa_qkv_b", "sa_proj_b", "ca_q_b", "ca_kv_b", "ca_proj_b",
        "fc_b", "fcp_b")] + [(np.arange(P) < HD).astype(np.float32) / DSCALE,
                             (np.arange(P) >= HD).astype(np.float32) / DSCALE])

    # causal mask, transposed layout: keep s<=t  ->  upper triangular [s,t]
    maskT = np.triu(np.ones((R, R), np.float32)).astype(wdt)

    weights = {k: np.ascontiguousarray(np.asarray(inputs[k]).astype(wdt))
               for k in ("sa_qkv_w", "sa_proj_w", "ca_q_w", "ca_kv_w",
                         "ca_proj_w", "fc_w", "fcp_w")}

    cpb = NC // B
    in_maps = []
    for c in range(NC):
        r0 = c * R
        pos = np.arange(r0, r0 + R) % T
        cos2 = np.vstack([cos[pos].T, cos[pos].T])          # [128, R]
        sin2 = np.vstack([sin[pos].T, sin[pos].T])          # [128, R]
        # fold rotate-half signs into sin: rows that multiply -x2
        sin2[0:HH, :] *= -1.0
        sin2[HD:HD + HH, :] *= -1.0
        cosB = np.ascontiguousarray(np.tile(cos2, (1, NPAIR)).astype(wdt))
        sinB = np.ascontiguousarray(np.tile(sin2, (1, NPAIR)).astype(wdt))
        b = c // cpb
        wpre = np.array([1.0 if (r // cpb == b and r < c) else 0.0
                         for r in range(NC)], np.float32)
        wtot = np.array([1.0 if r // cpb == b else 0.0
                         for r in range(NC)], np.float32)
        wsel = np.ascontiguousarray(
            np.tile(np.concatenate([wpre, wtot])[None, :], (P, 1)).astype(np.float32))
        m = dict(weights)
        m.update({
            "x_c": xf[r0:r0 + R].copy(),
            "m_c": mf[r0:r0 + R].copy(),
            "cosB": cosB, "sinB": sinB, "maskT": np.ascontiguousarray(maskT),
            "wsel": wsel, "params": params,
        })
        in_maps.append(m)
    return in_maps


# ---------------------------------------------------------------------------
# Bass program
# ---------------------------------------------------------------------------

def build_program(cfg: Cfg):
    import concourse.bass as bass
    import concourse.mybir as mybir
    import concourse.tile as tile
    from concourse import bacc
    from concourse.masks import make_identity
    from contextlib import ExitStack

    dt = mybir.dt
    f32 = dt.float32
    AF = mybir.ActivationFunctionType
    OP = mybir.AluOpType
    AX = mybir.AxisListType

    MMDT = {"fp16": dt.float16, "bf16": dt.bfloat16}[cfg.mm]
    WDT = MMDT

    B, T, C, H, NC = cfg.B, cfg.T, cfg.C, cfg.H, cfg.NCORE
    R, KC, NT, NPAIR, FR, AGW = cfg.R, cfg.KC, cfg.NT, cfg.NPAIR, cfg.FR, cfg.AGW
    assert R == 2 * P and NT == 2 and KC == 8 and NPAIR == 8
    GW = 4  # GEMM m-group width (PSUM banks)

    nc = bacc.Bacc("TRN2", target_bir_lowering=False, debug=False,
                   num_devices=cfg.NCORE)

    x_c = nc.dram_tensor("x_c", [R, C], MMDT, kind="ExternalInput")
    m_c = nc.dram_tensor("m_c", [R, C], MMDT, kind="ExternalInput")
    cosB_d = nc.dram_tensor("cosB", [P, FR], MMDT, kind="ExternalInput")
    sinB_d = nc.dram_tensor("sinB", [P, FR], MMDT, kind="ExternalInput")
    maskT_d = nc.dram_tensor("maskT", [R, R], MMDT, kind="ExternalInput")
    wsel_d = nc.dram_tensor("wsel", [P, 2 * NC], f32, kind="ExternalInput")
    NPCOL = 19 * KC + 2
    params_d = nc.dram_tensor("params", [P, NPCOL], f32, kind="ExternalInput")
    Wqkv = nc.dram_tensor("sa_qkv_w", [C, 3 * C], WDT, kind="ExternalInput")
    Wsap = nc.dram_tensor("sa_proj_w", [C, C], WDT, kind="ExternalInput")
    Wcaq = nc.dram_tensor("ca_q_w", [C, C], WDT, kind="ExternalInput")
    Wcakv = nc.dram_tensor("ca_kv_w", [C, 2 * C], WDT, kind="ExternalInput")
    Wcap = nc.dram_tensor("ca_proj_w", [C, C], WDT, kind="ExternalInput")
    Wfc = nc.dram_tensor("fc_w", [C, 4 * C], WDT, kind="ExternalInput")
    Wfcp = nc.dram_tensor("fcp_w", [4 * C, C], WDT, kind="ExternalInput")
    out_d = nc.dram_tensor("out", [R, C], MMDT, kind="ExternalOutput")

    off = {}
    cur = 0
    for pname, w in (("ln1_g", KC), ("ln1_b", KC), ("ln2_g", KC), ("ln2_b", KC),
                     ("ln3_g", KC), ("ln3_b", KC), ("qkv_b", 3 * KC),
                     ("sap_b", KC), ("caq_b", KC), ("cakv_b", 2 * KC),
                     ("cap_b", KC), ("fc_b", 4 * KC), ("fcp_b", KC),
                     ("m0h", 1), ("m1h", 1)):
        off[pname] = cur
        cur += w
    assert cur == NPCOL

    with tile.TileContext(nc) as tc, ExitStack() as ctx:
        const = ctx.enter_context(tc.tile_pool(name="const", bufs=1))
        act = ctx.enter_context(tc.tile_pool(name="act", bufs=1))
        wpool = ctx.enter_context(tc.tile_pool(name="wpool", bufs=12))
        tmp = ctx.enter_context(tc.tile_pool(name="tmp", bufs=2))
        gpool = ctx.enter_context(tc.tile_pool(name="gpool", bufs=GW, space="PSUM"))
        ypool = ctx.enter_context(tc.tile_pool(name="ypool", bufs=2, space="PSUM"))
        spool = ctx.enter_context(tc.tile_pool(name="spool", bufs=2, space="PSUM"))
        dram = ctx.enter_context(tc.tile_pool(name="dram", bufs=1, space="DRAM"))

        identf = const.tile([P, P], f32, name="identf")
        make_identity(nc, identf)
        ident = const.tile([P, P], MMDT, name="ident")
        nc.scalar.copy(ident[:], identf[:])
        params = const.tile([P, NPCOL], f32, name="params")
        wsel = const.tile([P, 2 * NC], f32, name="wsel")
        ones16 = const.tile([P, 1], MMDT, name="ones16")
        nc.vector.memset(ones16[:], 1.0)
        onesDS = const.tile([1, HD], MMDT, name="onesDS")
        nc.vector.memset(onesDS[:], 1.0 / DSCALE)
        epsT = const.tile([1, 1], f32, name="epsT")
        nc.vector.memset(epsT[:], LN_EPS)
        maskT = [const.tile([P, R], MMDT, name=f"maskT{n}") for n in range(NT)]
        cosT = const.tile([P, FR], MMDT, name="cosT")
        sinT = const.tile([P, FR], MMDT, name="sinT")

        def load_consts_late():
            # issued after the qkv weight stream: nothing here is needed
            # before the SA feature phase
            for n in range(NT):
                nc.sync.dma_start(maskT[n][:], maskT_d[n * P:(n + 1) * P, :])
            nc.sync.dma_start(cosT[:], cosB_d[:, :])
            nc.sync.dma_start(sinT[:], sinB_d[:, :])
            nc.sync.dma_start(wsel[:], wsel_d[:, :])

        def pcol(pname, j):
            return params[:, off[pname] + j:off[pname] + j + 1]

        # ---- load [R, C] natural -> KC transposed tiles [128, R] (XBAR) ----
        def load_transposed(src_dram, names, bufs=1):
            tiles = []
            for k in range(KC):
                t = act.tile([P, R], MMDT, name=names(k), bufs=bufs)
                nc.sync.dma_start_transpose(t[:], src_dram[:, k * P:(k + 1) * P])
                tiles.append(t)
            return tiles

        # ---- layernorm on transposed fp16 activations ----
        def layernorm(xt, gname, bname):
            ps = spool.tile([P, 512], f32, name="sps")
            sqs = []
            for k in range(KC):
                sq = tmp.tile([P, R], MMDT, name="lnsq", bufs=2)
                if k % 2 == 0:
                    nc.scalar.square(sq[:], xt[k][:])
                else:
                    nc.vector.tensor_mul(sq[:], xt[k][:], xt[k][:])
                sqs.append(sq)
            for k in range(KC):
                nc.tensor.matmul(ps[0:1, :R], lhsT=ones16[:], rhs=xt[k][:],
                                 start=(k == 0), stop=(k == KC - 1))
            for k in range(KC):
                nc.tensor.matmul(ps[32:33, :R], lhsT=ones16[:], rhs=sqs[k][:],
                                 start=(k == 0), stop=(k == KC - 1))
            mu = tmp.tile([1, R], f32, name="ln_mu", bufs=1)
            nc.scalar.mul(mu[:], ps[0:1, :R], 1.0 / C)
            musq = tmp.tile([1, R], f32, name="ln_musq", bufs=1)
            nc.scalar.square(musq[:], mu[:])
            var = tmp.tile([1, R], f32, name="ln_var", bufs=1)
            nc.vector.scalar_tensor_tensor(var[:], ps[32:33, :R], 1.0 / C,
                                           musq[:], op0=OP.mult,
                                           op1=OP.subtract)
            std = tmp.tile([1, R], f32, name="ln_std", bufs=1)
            nc.scalar.activation(std[:], var[:], AF.Sqrt, bias=epsT[:])
            rstd = tmp.tile([1, R], f32, name="ln_rstd", bufs=1)
            nc.vector.reciprocal_approx_fast(rstd[:], std[:])
            muh = tmp.tile([1, R], MMDT, name="ln_muh", bufs=1)
            nc.vector.tensor_copy(muh[:], mu[:])
            rstdh = tmp.tile([1, R], MMDT, name="ln_rstdh", bufs=1)
            nc.vector.tensor_copy(rstdh[:], rstd[:])
            mub = tmp.tile([P, R], MMDT, name="ln_mub", bufs=1)
            nc.gpsimd.partition_broadcast(mub[:], muh[:])
            rstdb = tmp.tile([P, R], MMDT, name="ln_rstdb", bufs=1)
            nc.gpsimd.partition_broadcast(rstdb[:], rstdh[:])
            hs = []
            for k in range(KC):
                t1 = tmp.tile([P, R], MMDT, name="ln_cen", bufs=2)
                if k % 2 == 0:
                    nc.gpsimd.tensor_sub(t1[:], xt[k][:], mub[:])
                else:
                    nc.vector.tensor_sub(t1[:], xt[k][:], mub[:])
                t2 = tmp.tile([P, R], MMDT, name="ln_nrm", bufs=2)
                nc.vector.tensor_mul(t2[:], t1[:], rstdb[:])
                h = act.tile([P, R], MMDT, name=f"h{k}", bufs=2)
                nc.vector.tensor_scalar(h[:], t2[:], pcol(gname, k),
                                        pcol(bname, k), op0=OP.mult, op1=OP.add)
                hs.append(h)
            return hs

        # ---- GEMM: out[M=F, N=R] = W[:,m]^T @ rhs, consumer per m-tile ----
        def gemm(w_dram, rhs_tiles, F, evict):
            KT = len(rhs_tiles)
            MT = F // P
            for g0 in range(0, MT, GW):
                pss = [gpool.tile([P, 512], f32, name="gps") for _ in range(GW)]
                for k in range(KT):
                    wt = wpool.tile([P, GW * P], WDT, name="wt")
                    nc.sync.dma_start(
                        wt[:], w_dram[k * P:(k + 1) * P, g0 * P:(g0 + GW) * P])
                    for j in range(GW):
                        nc.tensor.matmul(
                            pss[j][:, :R],
                            lhsT=wt[:, j * P:(j + 1) * P],
                            rhs=rhs_tiles[k][:],
                            start=(k == 0), stop=(k == KT - 1))
                for j in range(GW):
                    evict(g0 + j, pss[j])

        # ---- batched feature helpers ([128, FR] packed head-pair tiles) ----
        def elu1(src, oname):
            """elu(x)+1 = exp(min(x,0)) + max(x,0), batched over all pairs."""
            mn = tmp.tile([P, FR], MMDT, name="e_mn", bufs=1)
            nc.vector.tensor_scalar_min(mn[:], src[:], 0.0)
            nc.scalar.activation(mn[:], mn[:], AF.Exp)
            o = act.tile([P, FR], MMDT, name=oname)
            nc.vector.tensor_scalar_max(o[:], src[:], 0.0)
            nc.vector.tensor_add(o[:], o[:], mn[:])
            return o

        def rope(srcf, oname):
            """q*cos + rotate_half(q)*sin, sin rows pre-negated on host."""
            rot = tmp.tile([P, FR], MMDT, name="r_rot", bufs=1)
            nc.scalar.copy(rot[0:HH, :], srcf[HH:HD, :])
            nc.vector.tensor_copy(rot[HH:HD, :], srcf[0:HH, :])
            nc.scalar.copy(rot[HD:HD + HH, :], srcf[HD + HH:P, :])
            nc.vector.tensor_copy(rot[HD + HH:P, :], srcf[HD:HD + HH, :])
            nc.vector.tensor_mul(rot[:], rot[:], sinT[:])
            o = act.tile([P, FR], MMDT, name=oname)
            nc.vector.tensor_mul(o[:], srcf[:], cosT[:])
            nc.vector.tensor_add(o[:], o[:], rot[:])
            return o

        def kf_reduce(kfall, dst_tile, doff):
            """per-pair column sums of Kf -> dst_tile[:, doff:doff+NPAIR]."""
            try:
                grouped = kfall[:].rearrange("p (g t) -> p g t", g=NPAIR)
                nc.vector.reduce_sum(dst_tile[:, doff:doff + NPAIR], grouped,
                                     axis=AX.X)
            except Exception:
                for p_ in range(NPAIR):
                    nc.vector.reduce_sum(dst_tile[:, doff + p_:doff + p_ + 1],
                                         kfall[:, p_ * R:(p_ + 1) * R],
                                         axis=AX.X)

        def transpose_pairs(srcAll, oname):
            """[128(d), FR(s per pair)] -> [128(s within n), FR(p,n,d)]."""
            dst = act.tile([P, FR], MMDT, name=oname)
            for p in range(NPAIR):
                for n in range(NT):
                    pt = spool.tile([P, 512], MMDT, name="sps")
                    nc.tensor.transpose(
                        pt[:P, :P],
                        srcAll[:, p * R + n * P:p * R + (n + 1) * P],
                        ident[:, :])
                    d_ap = dst[:, p * R + n * P:p * R + (n + 1) * P]
                    if (p + n) % 2 == 0:
                        nc.scalar.copy(d_ap, pt[:P, :P])
                    else:
                        nc.vector.tensor_copy(d_ap, pt[:P, :P])
            return dst

        def kv_states(KnAll, VnAll, ag_tile):
            """per-pair state[d,e] = sum_s Kn[s,d] Vn[s,e] -> ag cols."""
            for p in range(NPAIR):
                st = spool.tile([P, 512], f32, name="sps")
                for h0 in (0, HD):
                    for n in range(NT):
                        base = p * R + n * P
                        nc.tensor.matmul(
                            st[h0:h0 + HD, :HD],
                            lhsT=KnAll[:, base + h0:base + h0 + HD],
                            rhs=VnAll[:, base + h0:base + h0 + HD],
                            start=(n == 0), stop=(n == NT - 1))
                if p % 2 == 0:
                    nc.scalar.copy(ag_tile[:, p * HD:(p + 1) * HD], st[:, :HD])
                else:
                    nc.vector.tensor_copy(ag_tile[:, p * HD:(p + 1) * HD],
                                          st[:, :HD])

        def denominators(QfAll, kf_tile, kf_off, oname):
            """denb [128, FR] fp16 = 1/(Qf . total-Kf), broadcast to rows.

            One 8-matmul accumulation chain into [16, R]: the lhsT for pair p
            is a [128, 16] block that is zero except col 2p (= kf, rows 0:64)
            and col 2p+1 (= kf, rows 64:128), so each chain step only touches
            its own two output rows."""
            kfblk = tmp.tile([P, 2 * NPAIR * NPAIR], MMDT, name="kfblk", bufs=2)
            nc.vector.memset(kfblk[:], 0.0)
            stepw = 2 * NPAIR + 2  # 18: block stride (16) + in-block col (2p)
            nc.vector.tensor_scalar_mul(
                kfblk[:, 0:(NPAIR - 1) * stepw + 1:stepw],
                kf_tile[:, kf_off:kf_off + NPAIR], pcol("m0h", 0))
            nc.vector.tensor_scalar_mul(
                kfblk[:, 1:(NPAIR - 1) * stepw + 2:stepw],
                kf_tile[:, kf_off:kf_off + NPAIR], pcol("m1h", 0))
            dps = spool.tile([P, 512], f32, name="sps")
            for p in range(NPAIR):
                nc.tensor.matmul(
                    dps[0:2 * NPAIR, :R],
                    lhsT=kfblk[:, 16 * p:16 * (p + 1)],
                    rhs=QfAll[:, p * R:(p + 1) * R],
                    start=(p == 0), stop=(p == NPAIR - 1))
            if cfg.debug_dump:
                dcp = tmp.tile([2 * NPAIR, R], f32, name="dcp", bufs=2)
                nc.scalar.copy(dcp[:], dps[0:2 * NPAIR, :R])
                dump(f"dps_{oname}", dcp[:])
                dump(f"kfblk_{oname}", kfblk[:])
            rcp32 = tmp.tile([2 * NPAIR, R], f32, name="rcp32", bufs=2)
            nc.vector.reciprocal_approx_fast(rcp32[:], dps[0:2 * NPAIR, :R])
            rcp = tmp.tile([2 * NPAIR, R], MMDT, name="rcp", bufs=2)
            nc.vector.tensor_copy(rcp[:], rcp32[:])
            if cfg.debug_dump:
                dump(f"rcp32_{oname}", rcp32[:])
            # flatten the 16 rows onto partition 0 (PE broadcast src row)
            rcpRow = tmp.tile([1, 2 * NPAIR * R], MMDT, name="rcpRow", bufs=1)
            nc.sync.dma_start(rcpRow[0:1, :], rcp[:, :])
            return rcpRow

        def bcast_denom(rcpRow, p):
            """[128, R] PSUM tile: rows h*64.. = rcp[2p+h]/DSCALE (fp32)."""
            bc = spool.tile([P, 512], f32, name="sps")
            for hi, h0 in enumerate((0, HD)):
                j = 2 * p + hi
                nc.tensor.matmul(bc[h0:h0 + HD, :R], lhsT=onesDS[0:1, :],
                                 rhs=rcpRow[0:1, j * R:(j + 1) * R],
                                 start=True, stop=True)
            return bc

        def dump(name, ap):
            if not cfg.debug_dump:
                return
            dd = nc.dram_tensor(f"dbg_{name}", list(ap.shape),
                                ap.dtype, kind="ExternalOutput")
            nc.sync.dma_start(dd[:, :], ap)

        # ============== phase 1: loads, qkv GEMM, SA features ==============

        xT = load_transposed(x_c, lambda k: f"res{k}", bufs=2)
        nc.sync.dma_start(params[:], params_d[:, :])

        QAll = act.tile([P, FR], MMDT, name="QAll")
        KAll = act.tile([P, FR], MMDT, name="KAll")
        VAll = act.tile([P, FR], MMDT, name="VAll")
        K2All = act.tile([P, FR], MMDT, name="K2All")
        V2All = act.tile([P, FR], MMDT, name="V2All")
        Q2All = act.tile([P, FR], MMDT, name="Q2All")

        def evict_to(dsts, bname):
            # dsts: list of (big_tile, first_m) section bounds
            def ev(m, ps):
                for tile_, lo, hi in dsts:
                    if lo <= m < hi:
                        dst = tile_[:, (m - lo) * R:(m - lo + 1) * R]
                        if m % 2 == 0:
                            nc.scalar.add(dst, ps[:, :R], pcol(bname, m))
                        else:
                            nc.vector.tensor_scalar(dst, ps[:, :R],
                                                    pcol(bname, m), None,
                                                    op0=OP.add)
                        return
                raise AssertionError(m)
            return ev

        h1 = layernorm(xT, "ln1_g", "ln1_b")
        gemm(Wqkv, h1, 3 * C,
             evict_to([(QAll, 0, 8), (KAll, 8, 16), (VAll, 16, 24)], "qkv_b"))
        load_consts_late()
        mT = load_transposed(m_c, lambda k: f"mm{k}")

        # SA K-side features + states
        ag_sa = act.tile([P, AGW], f32, name="ag_sa")
        dump('xT0', xT[0][:])
        dump('h10', h1[0][:])
        dump('QAll', QAll[:])
        dump('KAll', KAll[:])
        dump('VAll', VAll[:])
        Kf = elu1(KAll, "KfAll")
        kf_reduce(Kf, ag_sa, NPAIR * HD)
        Kr = rope(Kf, "KrAll")
        VnAll = transpose_pairs(VAll, "VnAll")
        KnAll = transpose_pairs(Kr, "KnAll")
        kv_states(KnAll, VnAll, ag_sa)
        dump('Kf', Kf[:])
        dump('Kr', Kr[:])
        dump('VnAll', VnAll[:])
        dump('ag_sa', ag_sa[:])

        # AllGather 1 (SA) starts while everything below proceeds
        ag1_in = dram.tile([P, AGW], f32, name="ag1_in")
        ag1_out = dram.tile([NC * P, AGW], f32, name="ag1_out",
                            addr_space="Shared")
        nc.sync.dma_start(ag1_in[:], ag_sa[:])
        nc.gpsimd.collective_compute(
            "AllGather", OP.bypass, replica_groups=[list(range(NC))],
            ins=[ag1_in[:].opt()], outs=[ag1_out[:].opt()])

        # ============== phase 2: cakv GEMM, CA features, AG2 ==============

        gemm(Wcakv, mT, 2 * C,
             evict_to([(K2All, 0, 8), (V2All, 8, 16)], "cakv_b"))

        ag_ca = act.tile([P, AGW], f32, name="ag_ca")
        K2f = elu1(K2All, "K2fAll")
        kf_reduce(K2f, ag_ca, NPAIR * HD)
        K2r = rope(K2f, "K2rAll")
        V2nAll = transpose_pairs(V2All, "V2nAll")
        K2nAll = transpose_pairs(K2r, "K2nAll")
        kv_states(K2nAll, V2nAll, ag_ca)

        ag2_in = dram.tile([P, AGW], f32, name="ag2_in")
        ag2_out = dram.tile([NC * P, AGW], f32, name="ag2_out",
                            addr_space="Shared")
        nc.sync.dma_start(ag2_in[:], ag_ca[:])
        nc.gpsimd.collective_compute(
            "AllGather", OP.bypass, replica_groups=[list(range(NC))],
            ins=[ag2_in[:].opt()], outs=[ag2_out[:].opt()])

        # ============== phase 3: Q features + intra-chunk attention =========

        Qf = elu1(QAll, "QfAll")
        Qr = rope(Qf, "QrAll")

        yiAll = act.tile([P, FR], MMDT, name="yiAll")
        for p in range(NPAIR):
            ypot = ypool.tile([P, 512], f32, name="ypot")
            for h0 in (0, HD):
                amss = []
                for n in range(NT):
                    pa = spool.tile([P, 512], f32, name="sps")
                    nc.tensor.matmul(
                        pa[:P, :R],
                        lhsT=Kr[h0:h0 + HD, p * R + n * P:p * R + (n + 1) * P],
                        rhs=Qr[h0:h0 + HD, p * R:(p + 1) * R],
                        start=True, stop=True)
                    am = tmp.tile([P, R], MMDT, name="attM", bufs=4)
                    nc.vector.tensor_mul(am[:], pa[:P, :R], maskT[n][:])
                    amss.append(am)
                for n in range(NT):
                    nc.tensor.matmul(
                        ypot[h0:h0 + HD, :R],
                        lhsT=VnAll[:, p * R + n * P + h0:p * R + n * P + h0 + HD],
                        rhs=amss[n][:],
                        start=(n == 0), stop=(n == NT - 1))
            if p % 2 == 0:
                nc.scalar.copy(yiAll[:, p * R:(p + 1) * R], ypot[:, :R])
            else:
                nc.vector.tensor_copy(yiAll[:, p * R:(p + 1) * R], ypot[:, :R])

        # ============== phase 4: AG1 readback, SA attention, sa_proj ========

        accP = act.tile([P, NPAIR * HD], f32, name="accP")
        accKf = act.tile([P, NPAIR], f32, name="accKf")
        nc.vector.memset(accP[:], 0.0)
        nc.vector.memset(accKf[:], 0.0)
        for r in range(NC):
            agr = tmp.tile([P, AGW], f32, name="agr", bufs=2)
            nc.sync.dma_start(agr[:], ag1_out[r * P:(r + 1) * P, :])
            nc.vector.scalar_tensor_tensor(
                accP[:], agr[:, :NPAIR * HD], wsel[:, r:r + 1], accP[:],
                op0=OP.mult, op1=OP.add)
            nc.vector.scalar_tensor_tensor(
                accKf[:], agr[:, NPAIR * HD:], wsel[:, NC + r:NC + r + 1],
                accKf[:], op0=OP.mult, op1=OP.add)
        accPm = act.tile([P, NPAIR * HD], MMDT, name="accPm")
        nc.vector.tensor_copy(accPm[:], accP[:])
        accKfm = act.tile([P, NPAIR], MMDT, name="accKfm")
        nc.vector.tensor_copy(accKfm[:], accKf[:])

        dump('accP', accP[:])
        dump('accKf', accKf[:])
        dump('Qf', Qf[:])
        dump('Qr', Qr[:])
        dump('yiAll', yiAll[:])
        rcpSA = denominators(Qf, accKfm, 0, "denb")
        yAll = act.tile([P, FR], MMDT, name="yAll")
        for p in range(NPAIR):
            bc = bcast_denom(rcpSA, p)
            stc = ypool.tile([P, 512], f32, name="ypot")
            for h0 in (0, HD):
                nc.tensor.matmul(
                    stc[h0:h0 + HD, :R],
                    lhsT=accPm[h0:h0 + HD, p * HD:(p + 1) * HD],
                    rhs=Qr[h0:h0 + HD, p * R:(p + 1) * R],
                    start=True, stop=True)
            ys = tmp.tile([P, R], MMDT, name="ysum", bufs=4)
            nc.vector.tensor_add(ys[:], stc[:, :R], yiAll[:, p * R:(p + 1) * R])
            nc.vector.tensor_mul(yAll[:, p * R:(p + 1) * R], ys[:], bc[:, :R])

        x1T = [None] * KC

        def evict_res(dst, bname, res, rname):
            def ev(m, ps):
                d = act.tile([P, R], MMDT, name=rname(m), bufs=2)
                nc.vector.scalar_tensor_tensor(d[:], ps[:, :R], pcol(bname, m),
                                               res[m][:], op0=OP.add, op1=OP.add)
                dst[m] = d
            return ev

        dump('yAll', yAll[:])
        yslc = [yAll[:, k * R:(k + 1) * R] for k in range(KC)]

        class _SL:
            def __init__(self, ap):
                self.ap = ap

            def __getitem__(self, s):
                return self.ap

        gemm(Wsap, [_SL(s) for s in yslc], C,
             evict_res(x1T, "sap_b", xT, lambda k: f"res{k}"))

        # ============== phase 5: cross attention ==============

        dump('x1T0', x1T[0][:])
        h2 = layernorm(x1T, "ln2_g", "ln2_b")
        gemm(Wcaq, h2, C, evict_to([(Q2All, 0, 8)], "caq_b"))
        Q2f = elu1(Q2All, "Q2fAll")
        Q2r = rope(Q2f, "Q2rAll")

        accC = act.tile([P, AGW], f32, name="accC")
        nc.vector.memset(accC[:], 0.0)
        for r in range(NC):
            agr = tmp.tile([P, AGW], f32, name="agr", bufs=2)
            nc.sync.dma_start(agr[:], ag2_out[r * P:(r + 1) * P, :])
            nc.vector.scalar_tensor_tensor(
                accC[:], agr[:], wsel[:, NC + r:NC + r + 1], accC[:],
                op0=OP.mult, op1=OP.add)
        accCm = act.tile([P, AGW], MMDT, name="accCm")
        nc.vector.tensor_copy(accCm[:], accC[:])

        rcpCA = denominators(Q2f, accCm, NPAIR * HD, "denb2")
        y2All = act.tile([P, FR], MMDT, name="y2All")
        for p in range(NPAIR):
            bc = bcast_denom(rcpCA, p)
            bcs = tmp.tile([P, R], f32, name="bcs", bufs=4)
            nc.vector.tensor_copy(bcs[:], bc[:, :R])
            stc = ypool.tile([P, 512], f32, name="ypot")
            for h0 in (0, HD):
                nc.tensor.matmul(
                    stc[h0:h0 + HD, :R],
                    lhsT=accCm[h0:h0 + HD, p * HD:(p + 1) * HD],
                    rhs=Q2r[h0:h0 + HD, p * R:(p + 1) * R],
                    start=True, stop=True)
            nc.vector.tensor_mul(y2All[:, p * R:(p + 1) * R], stc[:, :R],
                                 bcs[:])

        dump('y2All', y2All[:])
        x2T = [None] * KC
        y2slc = [y2All[:, k * R:(k + 1) * R] for k in range(KC)]
        gemm(Wcap, [_SL(s) for s in y2slc], C,
             evict_res(x2T, "cap_b", x1T, lambda k: f"res{k}"))

        # ============== phase 6: MLP ==============

        dump('x2T0', x2T[0][:])
        h3 = layernorm(x2T, "ln3_g", "ln3_b")
        gT = [None] * (4 * KC)

        def evict_gelu(m, ps):
            d = act.tile([P, R], MMDT, name=f"go{m}")
            nc.scalar.activation(d[:], ps[:, :R], AF.Gelu_apprx_tanh,
                                 bias=pcol("fc_b", m))
            gT[m] = d
        gemm(Wfc, h3, 4 * C, evict_gelu)

        xoT = [None] * KC
        gemm(Wfcp, gT, C, evict_res(xoT, "fcp_b", x2T, lambda k: f"res{k}"))

        # ============== transpose back + store ==============
        for n in range(NT):
            onat = tmp.tile([P, C], MMDT, name="nat", bufs=2)
            for k in range(KC):
                pool = ypool if k % 2 == 0 else spool
                pt = pool.tile([P, 512], MMDT,
                               name="ypot" if k % 2 == 0 else "sps")
                nc.tensor.transpose(pt[:P, :P],
                                    xoT[k][:, n * P:(n + 1) * P],
                                    ident[:, :])
                if k % 2 == 0:
                    nc.scalar.copy(onat[:, k * P:(k + 1) * P], pt[:P, :P])
                else:
                    nc.vector.tensor_copy(onat[:, k * P:(k + 1) * P],
                                          pt[:P, :P])
            nc.sync.dma_start(out_d[n * P:(n + 1) * P, :], onat[:, :])

    nc.compile()
    return nc


# ---------------------------------------------------------------------------
# Entry point
# ---------------------------------------------------------------------------

_CACHE = {}


def _get_program(cfg: Cfg):
    if cfg not in _CACHE:
        _CACHE[cfg] = build_program(cfg)
    return _CACHE[cfg]


def run(inputs, cfg: Cfg = Cfg(), trace: bool = False):
    from concourse.bass_utils import run_bass_kernel_spmd
    nc = _get_program(cfg)
    in_maps = _host_inputs(cfg, inputs)
    res = run_bass_kernel_spmd(nc, in_maps, core_ids=list(range(cfg.NCORE)),
                               trace=trace)
    outs = [np.asarray(res.results[c]["out"], np.float32)
            for c in range(cfg.NCORE)]
    full = np.concatenate(outs, axis=0).reshape(cfg.B, cfg.T, cfg.C)
    return np.ascontiguousarray(full), res


def kernel(**inputs):
    out, _ = run(inputs)
    return out


# revision 40
# speedup vs baseline: 1.1476x; 1.1476x over previous
"""Trainium2 Bass kernel for nn_DecoderBlock (linear-attention decoder block).

Sharding: token-parallel across 8 cores (each core owns (B*T)/8 = 256 rows of
the flattened [B*T, C] token stream; weights replicated per core). Linear
attention is exact: intra-chunk causal block plus cross-core KV prefix states
carried by two small AllGathers (SA states early, CA states later) so the
collectives overlap local compute. Activations stay transposed
([C partitions, tokens free]) so every GEMM lhsT is a plain DRAM weight slice.

Performance notes (vs the first working version):
 - whole datapath in fp16 (residuals included); inputs/outputs host-cast.
 - x/memory loaded with DMA-transpose (XBAR) instead of PE transposes.
 - all per-head-pair elementwise work batched into [128, 8*R] tiles
   (engine op cost scales with free-dim length, partitions are free).
 - denominators: 16 single-matmul chains into disjoint partition rows of one
   PSUM bank, one scalar-engine table reciprocal, 16 cheap gpsimd partition
   broadcasts (replaces 64 tiny matmuls + full-tile vector reciprocals).
 - PSUM statically partitioned: 4 banks GEMM, 2 banks attention, 2 scratch.
"""

import math
import numpy as np
from dataclasses import dataclass

P = 128
HD = 64  # head dim (fixed: C // n_head)
HH = HD // 2
LN_EPS = 1e-5
DSCALE = 4096.0  # denominator pre-scale: keeps 1/denom in fp16 normal range


@dataclass(frozen=True)
class Cfg:
    B: int = 2
    T: int = 1024
    C: int = 1024
    H: int = 16
    NCORE: int = 8
    mm: str = "fp16"  # GEMM dtype: fp16 | bf16
    gelu: str = "table"
    debug_dump: bool = False

    @property
    def R(self):
        return self.B * self.T // self.NCORE

    @property
    def KC(self):
        return self.C // P

    @property
    def NT(self):
        return math.ceil(self.R / P)

    @property
    def NPAIR(self):
        return self.H // 2

    @property
    def FR(self):
        return self.NPAIR * self.R

    @property
    def AGW(self):
        return HD * self.NPAIR + self.NPAIR  # 520: states + kf sums


# ---------------------------------------------------------------------------
# Host-side helpers
# ---------------------------------------------------------------------------

def _rope_tables(T):
    inv = 1.0 / (10000.0 ** (np.arange(0, HD, 2, dtype=np.float64) / HD))
    freqs = np.outer(np.arange(T), inv)
    emb = np.concatenate([freqs, freqs], axis=-1)
    return np.cos(emb).astype(np.float32), np.sin(emb).astype(np.float32)


def _pack_cols(vecs):
    flat = np.concatenate([np.asarray(v, np.float32).ravel() for v in vecs])
    assert flat.size % P == 0
    return np.ascontiguousarray(flat.reshape(-1, P).T)


def _np_wdt(mm):
    if mm == "fp16":
        return np.float16
    import ml_dtypes
    return ml_dtypes.bfloat16


def _host_inputs(cfg: Cfg, inputs):
    B, T, C, NC = cfg.B, cfg.T, cfg.C, cfg.NCORE
    R, NPAIR = cfg.R, cfg.NPAIR
    wdt = _np_wdt(cfg.mm)
    xf = np.ascontiguousarray(
        np.asarray(inputs["x"], np.float32).reshape(B * T, C).astype(wdt))
    mf = np.ascontiguousarray(
        np.asarray(inputs["memory"], np.float32).reshape(B * T, C).astype(wdt))
    cos, sin = _rope_tables(T)

    params = _pack_cols([inputs[k] for k in (
        "ln1_g", "ln1_b", "ln2_g", "ln2_b", "ln3_g", "ln3_b",
        "sa_qkv_b", "sa_proj_b", "ca_q_b", "ca_kv_b", "ca_proj_b",
        "fc_b", "fcp_b")] + [(np.arange(P) < HD).astype(np.float32) / DSCALE,
                             (np.arange(P) >= HD).astype(np.float32) / DSCALE])

    # causal mask, transposed layout: keep s<=t  ->  upper triangular [s,t]
    maskT = np.triu(np.ones((R, R), np.float32)).astype(wdt)

    weights = {k: np.ascontiguousarray(np.asarray(inputs[k]).astype(wdt))
               for k in ("sa_qkv_w", "sa_proj_w", "ca_q_w", "ca_kv_w",
                         "ca_proj_w", "fc_w", "fcp_w")}

    cpb = NC // B
    in_maps = []
    for c in range(NC):
        r0 = c * R
        pos = np.arange(r0, r0 + R) % T
        cos2 = np.vstack([cos[pos].T, cos[pos].T])          # [128, R]
        sin2 = np.vstack([sin[pos].T, sin[pos].T])          # [128, R]
        # fold rotate-half signs into sin: rows that multiply -x2
        sin2[0:HH, :] *= -1.0
        sin2[HD:HD + HH, :] *= -1.0
        cosB = np.ascontiguousarray(np.tile(cos2, (1, NPAIR)).astype(wdt))
        sinB = np.ascontiguousarray(np.tile(sin2, (1, NPAIR)).astype(wdt))
        b = c // cpb
        wpre = np.array([1.0 if (r // cpb == b and r < c) else 0.0
                         for r in range(NC)], np.float32)
        wtot = np.array([1.0 if r // cpb == b else 0.0
                         for r in range(NC)], np.float32)
        wsel = np.ascontiguousarray(
            np.tile(np.concatenate([wpre, wtot])[None, :], (P, 1)).astype(np.float32))
        m = dict(weights)
        m.update({
            "x_c": xf[r0:r0 + R].copy(),
            "m_c": mf[r0:r0 + R].copy(),
            "cosB": cosB, "sinB": sinB, "maskT": np.ascontiguousarray(maskT),
            "wsel": wsel, "params": params,
        })
        in_maps.append(m)
    return in_maps


# ---------------------------------------------------------------------------
# Bass program
# ---------------------------------------------------------------------------

def build_program(cfg: Cfg):
    import concourse.bass as bass
    import concourse.mybir as mybir
    import concourse.tile as tile
    from concourse import bacc
    from concourse.masks import make_identity
    from contextlib import ExitStack

    dt = mybir.dt
    f32 = dt.float32
    AF = mybir.ActivationFunctionType
    OP = mybir.AluOpType
    AX = mybir.AxisListType

    MMDT = {"fp16": dt.float16, "bf16": dt.bfloat16}[cfg.mm]
    WDT = MMDT

    B, T, C, H, NC = cfg.B, cfg.T, cfg.C, cfg.H, cfg.NCORE
    R, KC, NT, NPAIR, FR, AGW = cfg.R, cfg.KC, cfg.NT, cfg.NPAIR, cfg.FR, cfg.AGW
    assert R == 2 * P and NT == 2 and KC == 8 and NPAIR == 8
    GW = 4  # GEMM m-group width (PSUM banks)

    nc = bacc.Bacc("TRN2", target_bir_lowering=False, debug=False,
                   num_devices=cfg.NCORE)

    x_c = nc.dram_tensor("x_c", [R, C], MMDT, kind="ExternalInput")
    m_c = nc.dram_tensor("m_c", [R, C], MMDT, kind="ExternalInput")
    cosB_d = nc.dram_tensor("cosB", [P, FR], MMDT, kind="ExternalInput")
    sinB_d = nc.dram_tensor("sinB", [P, FR], MMDT, kind="ExternalInput")
    maskT_d = nc.dram_tensor("maskT", [R, R], MMDT, kind="ExternalInput")
    wsel_d = nc.dram_tensor("wsel", [P, 2 * NC], f32, kind="ExternalInput")
    NPCOL = 19 * KC + 2
    params_d = nc.dram_tensor("params", [P, NPCOL], f32, kind="ExternalInput")
    Wqkv = nc.dram_tensor("sa_qkv_w", [C, 3 * C], WDT, kind="ExternalInput")
    Wsap = nc.dram_tensor("sa_proj_w", [C, C], WDT, kind="ExternalInput")
    Wcaq = nc.dram_tensor("ca_q_w", [C, C], WDT, kind="ExternalInput")
    Wcakv = nc.dram_tensor("ca_kv_w", [C, 2 * C], WDT, kind="ExternalInput")
    Wcap = nc.dram_tensor("ca_proj_w", [C, C], WDT, kind="ExternalInput")
    Wfc = nc.dram_tensor("fc_w", [C, 4 * C], WDT, kind="ExternalInput")
    Wfcp = nc.dram_tensor("fcp_w", [4 * C, C], WDT, kind="ExternalInput")
    out_d = nc.dram_tensor("out", [R, C], MMDT, kind="ExternalOutput")

    off = {}
    cur = 0
    for pname, w in (("ln1_g", KC), ("ln1_b", KC), ("ln2_g", KC), ("ln2_b", KC),
                     ("ln3_g", KC), ("ln3_b", KC), ("qkv_b", 3 * KC),
                     ("sap_b", KC), ("caq_b", KC), ("cakv_b", 2 * KC),
                     ("cap_b", KC), ("fc_b", 4 * KC), ("fcp_b", KC),
                     ("m0h", 1), ("m1h", 1)):
        off[pname] = cur
        cur += w
    assert cur == NPCOL

    with tile.TileContext(nc) as tc, ExitStack() as ctx:
        const = ctx.enter_context(tc.tile_pool(name="const", bufs=1))
        act = ctx.enter_context(tc.tile_pool(name="act", bufs=1))
        wpool = ctx.enter_context(tc.tile_pool(name="wpool", bufs=12))
        tmp = ctx.enter_context(tc.tile_pool(name="tmp", bufs=2))
        gpool = ctx.enter_context(tc.tile_pool(name="gpool", bufs=GW, space="PSUM"))
        ypool = ctx.enter_context(tc.tile_pool(name="ypool", bufs=2, space="PSUM"))
        spool = ctx.enter_context(tc.tile_pool(name="spool", bufs=2, space="PSUM"))
        dram = ctx.enter_context(tc.tile_pool(name="dram", bufs=1, space="DRAM"))

        identf = const.tile([P, P], f32, name="identf")
        make_identity(nc, identf)
        ident = const.tile([P, P], MMDT, name="ident")
        nc.scalar.copy(ident[:], identf[:])
        params = const.tile([P, NPCOL], f32, name="params")
        wsel = const.tile([P, 2 * NC], f32, name="wsel")
        ones16 = const.tile([P, 1], MMDT, name="ones16")
        nc.vector.memset(ones16[:], 1.0)
        onesDS = const.tile([1, HD], MMDT, name="onesDS")
        nc.vector.memset(onesDS[:], 1.0 / DSCALE)
        epsT = const.tile([1, 1], f32, name="epsT")
        nc.vector.memset(epsT[:], LN_EPS)
        maskT = [const.tile([P, R], MMDT, name=f"maskT{n}") for n in range(NT)]
        cosT = const.tile([P, FR], MMDT, name="cosT")
        sinT = const.tile([P, FR], MMDT, name="sinT")

        def load_consts_late():
            # issued after the qkv weight stream: nothing here is needed
            # before the SA feature phase
            for n in range(NT):
                nc.sync.dma_start(maskT[n][:], maskT_d[n * P:(n + 1) * P, :])
            nc.sync.dma_start(cosT[:], cosB_d[:, :])
            nc.sync.dma_start(sinT[:], sinB_d[:, :])
            nc.sync.dma_start(wsel[:], wsel_d[:, :])

        def pcol(pname, j):
            return params[:, off[pname] + j:off[pname] + j + 1]

        # ---- load [R, C] natural -> KC transposed tiles [128, R] (XBAR) ----
        def load_transposed(src_dram, names, bufs=1):
            tiles = []
            for k in range(KC):
                t = act.tile([P, R], MMDT, name=names(k), bufs=bufs)
                nc.sync.dma_start_transpose(t[:], src_dram[:, k * P:(k + 1) * P])
                tiles.append(t)
            return tiles

        # ---- layernorm on transposed fp16 activations ----
        def layernorm(xt, gname, bname):
            ps = spool.tile([P, 512], f32, name="sps")
            sqs = []
            for k in range(KC):
                sq = tmp.tile([P, R], MMDT, name="lnsq", bufs=2)
                if k % 2 == 0:
                    nc.scalar.square(sq[:], xt[k][:])
                else:
                    nc.vector.tensor_mul(sq[:], xt[k][:], xt[k][:])
                sqs.append(sq)
            for k in range(KC):
                nc.tensor.matmul(ps[0:1, :R], lhsT=ones16[:], rhs=xt[k][:],
                                 start=(k == 0), stop=(k == KC - 1))
            for k in range(KC):
                nc.tensor.matmul(ps[32:33, :R], lhsT=ones16[:], rhs=sqs[k][:],
                                 start=(k == 0), stop=(k == KC - 1))
            mu = tmp.tile([1, R], f32, name="ln_mu", bufs=1)
            nc.scalar.mul(mu[:], ps[0:1, :R], 1.0 / C)
            musq = tmp.tile([1, R], f32, name="ln_musq", bufs=1)
            nc.scalar.square(musq[:], mu[:])
            var = tmp.tile([1, R], f32, name="ln_var", bufs=1)
            nc.vector.scalar_tensor_tensor(var[:], ps[32:33, :R], 1.0 / C,
                                           musq[:], op0=OP.mult,
                                           op1=OP.subtract)
            std = tmp.tile([1, R], f32, name="ln_std", bufs=1)
            nc.scalar.activation(std[:], var[:], AF.Sqrt, bias=epsT[:])
            rstd = tmp.tile([1, R], f32, name="ln_rstd", bufs=1)
            nc.vector.reciprocal_approx_fast(rstd[:], std[:])
            muh = tmp.tile([1, R], MMDT, name="ln_muh", bufs=1)
            nc.vector.tensor_copy(muh[:], mu[:])
            rstdh = tmp.tile([1, R], MMDT, name="ln_rstdh", bufs=1)
            nc.vector.tensor_copy(rstdh[:], rstd[:])
            mub = tmp.tile([P, R], MMDT, name="ln_mub", bufs=1)
            nc.gpsimd.partition_broadcast(mub[:], muh[:])
            rstdb = tmp.tile([P, R], MMDT, name="ln_rstdb", bufs=1)
            nc.gpsimd.partition_broadcast(rstdb[:], rstdh[:])
            hs = []
            for k in range(KC):
                t1 = tmp.tile([P, R], MMDT, name="ln_cen", bufs=2)
                if k % 2 == 0:
                    nc.gpsimd.tensor_sub(t1[:], xt[k][:], mub[:])
                else:
                    nc.vector.tensor_sub(t1[:], xt[k][:], mub[:])
                t2 = tmp.tile([P, R], MMDT, name="ln_nrm", bufs=2)
                nc.vector.tensor_mul(t2[:], t1[:], rstdb[:])
                h = act.tile([P, R], MMDT, name=f"h{k}", bufs=2)
                nc.vector.tensor_scalar(h[:], t2[:], pcol(gname, k),
                                        pcol(bname, k), op0=OP.mult, op1=OP.add)
                hs.append(h)
            return hs

        # ---- GEMM: out[M=F, N=R] = W[:,m]^T @ rhs, consumer per m-tile ----
        def gemm(w_dram, rhs_tiles, F, evict):
            KT = len(rhs_tiles)
            MT = F // P
            for g0 in range(0, MT, GW):
                pss = [gpool.tile([P, 512], f32, name="gps") for _ in range(GW)]
                for k in range(KT):
                    wt = wpool.tile([P, GW * P], WDT, name="wt")
                    nc.sync.dma_start(
                        wt[:], w_dram[k * P:(k + 1) * P, g0 * P:(g0 + GW) * P])
                    for j in range(GW):
                        nc.tensor.matmul(
                            pss[j][:, :R],
                            lhsT=wt[:, j * P:(j + 1) * P],
                            rhs=rhs_tiles[k][:],
                            start=(k == 0), stop=(k == KT - 1))
                for j in range(GW):
                    evict(g0 + j, pss[j])

        # ---- batched feature helpers ([128, FR] packed head-pair tiles) ----
        def elu1(src, oname):
            """elu(x)+1 = exp(min(x,0)) + max(x,0), batched over all pairs."""
            mn = tmp.tile([P, FR], MMDT, name="e_mn", bufs=1)
            nc.vector.tensor_scalar_min(mn[:], src[:], 0.0)
            nc.scalar.activation(mn[:], mn[:], AF.Exp)
            o = act.tile([P, FR], MMDT, name=oname)
            nc.vector.tensor_scalar_max(o[:], src[:], 0.0)
            nc.vector.tensor_add(o[:], o[:], mn[:])
            return o

        def rope(srcf, oname):
            """q*cos + rotate_half(q)*sin, sin rows pre-negated on host."""
            rot = tmp.tile([P, FR], MMDT, name="r_rot", bufs=1)
            nc.scalar.copy(rot[0:HH, :], srcf[HH:HD, :])
            nc.vector.tensor_copy(rot[HH:HD, :], srcf[0:HH, :])
            nc.scalar.copy(rot[HD:HD + HH, :], srcf[HD + HH:P, :])
            nc.vector.tensor_copy(rot[HD + HH:P, :], srcf[HD:HD + HH, :])
            nc.vector.tensor_mul(rot[:], rot[:], sinT[:])
            o = act.tile([P, FR], MMDT, name=oname)
            nc.vector.tensor_mul(o[:], srcf[:], cosT[:])
            nc.vector.tensor_add(o[:], o[:], rot[:])
            return o

        def kf_reduce(kfall, dst_tile, doff):
            """per-pair column sums of Kf -> dst_tile[:, doff:doff+NPAIR]."""
            try:
                grouped = kfall[:].rearrange("p (g t) -> p g t", g=NPAIR)
                nc.vector.reduce_sum(dst_tile[:, doff:doff + NPAIR], grouped,
                                     axis=AX.X)
            except Exception:
                for p_ in range(NPAIR):
                    nc.vector.reduce_sum(dst_tile[:, doff + p_:doff + p_ + 1],
                                         kfall[:, p_ * R:(p_ + 1) * R],
                                         axis=AX.X)

        def transpose_pairs(srcAll, oname):
            """[128(d), FR(s per pair)] -> [128(s within n), FR(p,n,d)]."""
            dst = act.tile([P, FR], MMDT, name=oname)
            for p in range(NPAIR):
                for n in range(NT):
                    pt = spool.tile([P, 512], MMDT, name="sps")
                    nc.tensor.transpose(
                        pt[:P, :P],
                        srcAll[:, p * R + n * P:p * R + (n + 1) * P],
                        ident[:, :])
                    d_ap = dst[:, p * R + n * P:p * R + (n + 1) * P]
                    if (p + n) % 2 == 0:
                        nc.scalar.copy(d_ap, pt[:P, :P])
                    else:
                        nc.vector.tensor_copy(d_ap, pt[:P, :P])
            return dst

        def kv_states(KnAll, VnAll, ag_tile):
            """per-pair state[d,e] = sum_s Kn[s,d] Vn[s,e] -> ag cols."""
            for p in range(NPAIR):
                st = spool.tile([P, 512], f32, name="sps")
                for h0 in (0, HD):
                    for n in range(NT):
                        base = p * R + n * P
                        nc.tensor.matmul(
                            st[h0:h0 + HD, :HD],
                            lhsT=KnAll[:, base + h0:base + h0 + HD],
                            rhs=VnAll[:, base + h0:base + h0 + HD],
                            start=(n == 0), stop=(n == NT - 1))
                if p % 2 == 0:
                    nc.scalar.copy(ag_tile[:, p * HD:(p + 1) * HD], st[:, :HD])
                else:
                    nc.vector.tensor_copy(ag_tile[:, p * HD:(p + 1) * HD],
                                          st[:, :HD])

        def denominators(QfAll, kf_tile, kf_off, oname):
            """denb [128, FR] fp16 = 1/(Qf . total-Kf), broadcast to rows.

            One 8-matmul accumulation chain into [16, R]: the lhsT for pair p
            is a [128, 16] block that is zero except col 2p (= kf, rows 0:64)
            and col 2p+1 (= kf, rows 64:128), so each chain step only touches
            its own two output rows."""
            kfblk = tmp.tile([P, 2 * NPAIR * NPAIR], MMDT, name="kfblk", bufs=2)
            nc.vector.memset(kfblk[:], 0.0)
            stepw = 2 * NPAIR + 2  # 18: block stride (16) + in-block col (2p)
            nc.vector.tensor_scalar_mul(
                kfblk[:, 0:(NPAIR - 1) * stepw + 1:stepw],
                kf_tile[:, kf_off:kf_off + NPAIR], pcol("m0h", 0))
            nc.vector.tensor_scalar_mul(
                kfblk[:, 1:(NPAIR - 1) * stepw + 2:stepw],
                kf_tile[:, kf_off:kf_off + NPAIR], pcol("m1h", 0))
            dps = spool.tile([P, 512], f32, name="sps")
            for p in range(NPAIR):
                nc.tensor.matmul(
                    dps[0:2 * NPAIR, :R],
                    lhsT=kfblk[:, 16 * p:16 * (p + 1)],
                    rhs=QfAll[:, p * R:(p + 1) * R],
                    start=(p == 0), stop=(p == NPAIR - 1))
            if cfg.debug_dump:
                dcp = tmp.tile([2 * NPAIR, R], f32, name="dcp", bufs=2)
                nc.scalar.copy(dcp[:], dps[0:2 * NPAIR, :R])
                dump(f"dps_{oname}", dcp[:])
                dump(f"kfblk_{oname}", kfblk[:])
            rcp32 = tmp.tile([2 * NPAIR, R], f32, name="rcp32", bufs=2)
            nc.vector.reciprocal_approx_fast(rcp32[:], dps[0:2 * NPAIR, :R])
            rcp = tmp.tile([2 * NPAIR, R], MMDT, name="rcp", bufs=2)
            nc.vector.tensor_copy(rcp[:], rcp32[:])
            if cfg.debug_dump:
                dump(f"rcp32_{oname}", rcp32[:])
            # flatten the 16 rows onto partition 0 (PE broadcast src row)
            rcpRow = tmp.tile([1, 2 * NPAIR * R], MMDT, name="rcpRow", bufs=1)
            nc.sync.dma_start(rcpRow[0:1, :], rcp[:, :])
            return rcpRow

        def bcast_denom(rcpRow, p):
            """[128, R] PSUM tile: rows h*64.. = rcp[2p+h]/DSCALE (fp32)."""
            bc = spool.tile([P, 512], f32, name="sps")
            for hi, h0 in enumerate((0, HD)):
                j = 2 * p + hi
                nc.tensor.matmul(bc[h0:h0 + HD, :R], lhsT=onesDS[0:1, :],
                                 rhs=rcpRow[0:1, j * R:(j + 1) * R],
                                 start=True, stop=True)
            return bc

        def dump(name, ap):
            if not cfg.debug_dump:
                return
            dd = nc.dram_tensor(f"dbg_{name}", list(ap.shape),
                                ap.dtype, kind="ExternalOutput")
            nc.sync.dma_start(dd[:, :], ap)

        # ============== phase 1: loads, qkv GEMM, SA features ==============

        xT = load_transposed(x_c, lambda k: f"res{k}", bufs=2)
        nc.sync.dma_start(params[:], params_d[:, :])
        load_consts_late()

        QAll = act.tile([P, FR], MMDT, name="QAll")
        KAll = act.tile([P, FR], MMDT, name="KAll")
        VAll = act.tile([P, FR], MMDT, name="VAll")
        K2All = act.tile([P, FR], MMDT, name="K2All")
        V2All = act.tile([P, FR], MMDT, name="V2All")
        Q2All = act.tile([P, FR], MMDT, name="Q2All")

        def evict_to(dsts, bname):
            # dsts: list of (big_tile, first_m) section bounds
            def ev(m, ps):
                for tile_, lo, hi in dsts:
                    if lo <= m < hi:
                        dst = tile_[:, (m - lo) * R:(m - lo + 1) * R]
                        if m % 2 == 0:
                            nc.scalar.add(dst, ps[:, :R], pcol(bname, m))
                        else:
                            nc.vector.tensor_scalar(dst, ps[:, :R],
                                                    pcol(bname, m), None,
                                                    op0=OP.add)
                        return
                raise AssertionError(m)
            return ev

        h1 = layernorm(xT, "ln1_g", "ln1_b")
        gemm(Wqkv, h1, 3 * C,
             evict_to([(QAll, 0, 8), (KAll, 8, 16), (VAll, 16, 24)], "qkv_b"))
        mT = load_transposed(m_c, lambda k: f"mm{k}")

        # SA K-side features + states
        ag_sa = act.tile([P, AGW], f32, name="ag_sa")
        dump('xT0', xT[0][:])
        dump('h10', h1[0][:])
        dump('QAll', QAll[:])
        dump('KAll', KAll[:])
        dump('VAll', VAll[:])
        Kf = elu1(KAll, "KfAll")
        kf_reduce(Kf, ag_sa, NPAIR * HD)
        Kr = rope(Kf, "KrAll")
        VnAll = transpose_pairs(VAll, "VnAll")
        KnAll = transpose_pairs(Kr, "KnAll")
        kv_states(KnAll, VnAll, ag_sa)
        dump('Kf', Kf[:])
        dump('Kr', Kr[:])
        dump('VnAll', VnAll[:])
        dump('ag_sa', ag_sa[:])

        # AllGather 1 (SA) starts while everything below proceeds
        ag1_in = dram.tile([P, AGW], f32, name="ag1_in")
        ag1_out = dram.tile([NC * P, AGW], f32, name="ag1_out",
                            addr_space="Shared")
        nc.sync.dma_start(ag1_in[:], ag_sa[:])
        nc.gpsimd.collective_compute(
            "AllGather", OP.bypass, replica_groups=[list(range(NC))],
            ins=[ag1_in[:].opt()], outs=[ag1_out[:].opt()])

        # ============== phase 2: cakv GEMM, CA features, AG2 ==============

        gemm(Wcakv, mT, 2 * C,
             evict_to([(K2All, 0, 8), (V2All, 8, 16)], "cakv_b"))

        ag_ca = act.tile([P, AGW], f32, name="ag_ca")
        K2f = elu1(K2All, "K2fAll")
        kf_reduce(K2f, ag_ca, NPAIR * HD)
        K2r = rope(K2f, "K2rAll")
        V2nAll = transpose_pairs(V2All, "V2nAll")
        K2nAll = transpose_pairs(K2r, "K2nAll")
        kv_states(K2nAll, V2nAll, ag_ca)

        ag2_in = dram.tile([P, AGW], f32, name="ag2_in")
        ag2_out = dram.tile([NC * P, AGW], f32, name="ag2_out",
                            addr_space="Shared")
        nc.sync.dma_start(ag2_in[:], ag_ca[:])
        nc.gpsimd.collective_compute(
            "AllGather", OP.bypass, replica_groups=[list(range(NC))],
            ins=[ag2_in[:].opt()], outs=[ag2_out[:].opt()])

        # ============== phase 3: Q features + intra-chunk attention =========

        Qf = elu1(QAll, "QfAll")
        Qr = rope(Qf, "QrAll")

        yiAll = act.tile([P, FR], MMDT, name="yiAll")
        for p in range(NPAIR):
            ypot = ypool.tile([P, 512], f32, name="ypot")
            for h0 in (0, HD):
                amss = []
                for n in range(NT):
                    pa = spool.tile([P, 512], f32, name="sps")
                    nc.tensor.matmul(
                        pa[:P, :R],
                        lhsT=Kr[h0:h0 + HD, p * R + n * P:p * R + (n + 1) * P],
                        rhs=Qr[h0:h0 + HD, p * R:(p + 1) * R],
                        start=True, stop=True)
                    am = tmp.tile([P, R], MMDT, name="attM", bufs=4)
                    nc.vector.tensor_mul(am[:], pa[:P, :R], maskT[n][:])
                    amss.append(am)
                for n in range(NT):
                    nc.tensor.matmul(
                        ypot[h0:h0 + HD, :R],
                        lhsT=VnAll[:, p * R + n * P + h0:p * R + n * P + h0 + HD],
                        rhs=amss[n][:],
                        start=(n == 0), stop=(n == NT - 1))
            if p % 2 == 0:
                nc.scalar.copy(yiAll[:, p * R:(p + 1) * R], ypot[:, :R])
            else:
                nc.vector.tensor_copy(yiAll[:, p * R:(p + 1) * R], ypot[:, :R])

        # ============== phase 4: AG1 readback, SA attention, sa_proj ========

        accP = act.tile([P, NPAIR * HD], f32, name="accP")
        accKf = act.tile([P, NPAIR], f32, name="accKf")
        nc.vector.memset(accP[:], 0.0)
        nc.vector.memset(accKf[:], 0.0)
        for r in range(NC):
            agr = tmp.tile([P, AGW], f32, name="agr", bufs=2)
            nc.sync.dma_start(agr[:], ag1_out[r * P:(r + 1) * P, :])
            nc.vector.scalar_tensor_tensor(
                accP[:], agr[:, :NPAIR * HD], wsel[:, r:r + 1], accP[:],
                op0=OP.mult, op1=OP.add)
            nc.vector.scalar_tensor_tensor(
                accKf[:], agr[:, NPAIR * HD:], wsel[:, NC + r:NC + r + 1],
                accKf[:], op0=OP.mult, op1=OP.add)
        accPm = act.tile([P, NPAIR * HD], MMDT, name="accPm")
        nc.vector.tensor_copy(accPm[:], accP[:])
        accKfm = act.tile([P, NPAIR], MMDT, name="accKfm")
        nc.vector.tensor_copy(accKfm[:], accKf[:])

        dump('accP', accP[:])
        dump('accKf', accKf[:])
        dump('Qf', Qf[:])
        dump('Qr', Qr[:])
        dump('yiAll', yiAll[:])
        rcpSA = denominators(Qf, accKfm, 0, "denb")
        yAll = act.tile([P, FR], MMDT, name="yAll")
        for p in range(NPAIR):
            bc = bcast_denom(rcpSA, p)
            stc = ypool.tile([P, 512], f32, name="ypot")
            for h0 in (0, HD):
                nc.tensor.matmul(
                    stc[h0:h0 + HD, :R],
                    lhsT=accPm[h0:h0 + HD, p * HD:(p + 1) * HD],
                    rhs=Qr[h0:h0 + HD, p * R:(p + 1) * R],
                    start=True, stop=True)
            ys = tmp.tile([P, R], MMDT, name="ysum", bufs=4)
            nc.vector.tensor_add(ys[:], stc[:, :R], yiAll[:, p * R:(p + 1) * R])
            nc.vector.tensor_mul(yAll[:, p * R:(p + 1) * R], ys[:], bc[:, :R])

        x1T = [None] * KC

        def evict_res(dst, bname, res, rname):
            def ev(m, ps):
                d = act.tile([P, R], MMDT, name=rname(m), bufs=2)
                nc.vector.scalar_tensor_tensor(d[:], ps[:, :R], pcol(bname, m),
                                               res[m][:], op0=OP.add, op1=OP.add)
                dst[m] = d
            return ev

        dump('yAll', yAll[:])
        yslc = [yAll[:, k * R:(k + 1) * R] for k in range(KC)]

        class _SL:
            def __init__(self, ap):
                self.ap = ap

            def __getitem__(self, s):
                return self.ap

        gemm(Wsap, [_SL(s) for s in yslc], C,
             evict_res(x1T, "sap_b", xT, lambda k: f"res{k}"))

        # ============== phase 5: cross attention ==============

        dump('x1T0', x1T[0][:])
        h2 = layernorm(x1T, "ln2_g", "ln2_b")
        gemm(Wcaq, h2, C, evict_to([(Q2All, 0, 8)], "caq_b"))
        Q2f = elu1(Q2All, "Q2fAll")
        Q2r = rope(Q2f, "Q2rAll")

        accC = act.tile([P, AGW], f32, name="accC")
        nc.vector.memset(accC[:], 0.0)
        for r in range(NC):
            agr = tmp.tile([P, AGW], f32, name="agr", bufs=2)
            nc.sync.dma_start(agr[:], ag2_out[r * P:(r + 1) * P, :])
            nc.vector.scalar_tensor_tensor(
                accC[:], agr[:], wsel[:, NC + r:NC + r + 1], accC[:],
                op0=OP.mult, op1=OP.add)
        accCm = act.tile([P, AGW], MMDT, name="accCm")
        nc.vector.tensor_copy(accCm[:], accC[:])

        rcpCA = denominators(Q2f, accCm, NPAIR * HD, "denb2")
        y2All = act.tile([P, FR], MMDT, name="y2All")
        for p in range(NPAIR):
            bc = bcast_denom(rcpCA, p)
            bcs = tmp.tile([P, R], f32, name="bcs", bufs=4)
            nc.vector.tensor_copy(bcs[:], bc[:, :R])
            stc = ypool.tile([P, 512], f32, name="ypot")
            for h0 in (0, HD):
                nc.tensor.matmul(
                    stc[h0:h0 + HD, :R],
                    lhsT=accCm[h0:h0 + HD, p * HD:(p + 1) * HD],
                    rhs=Q2r[h0:h0 + HD, p * R:(p + 1) * R],
                    start=True, stop=True)
            nc.vector.tensor_mul(y2All[:, p * R:(p + 1) * R], stc[:, :R],
                                 bcs[:])

        dump('y2All', y2All[:])
        x2T = [None] * KC
        y2slc = [y2All[:, k * R:(k + 1) * R] for k in range(KC)]
        gemm(Wcap, [_SL(s) for s in y2slc], C,
             evict_res(x2T, "cap_b", x1T, lambda k: f"res{k}"))

        # ============== phase 6: MLP ==============

        dump('x2T0', x2T[0][:])
        h3 = layernorm(x2T, "ln3_g", "ln3_b")
        gT = [None] * (4 * KC)

        def evict_gelu(m, ps):
            d = act.tile([P, R], MMDT, name=f"go{m}")
            nc.scalar.activation(d[:], ps[:, :R], AF.Gelu_apprx_tanh,
                                 bias=pcol("fc_b", m))
            gT[m] = d
        gemm(Wfc, h3, 4 * C, evict_gelu)

        xoT = [None] * KC
        gemm(Wfcp, gT, C, evict_res(xoT, "fcp_b", x2T, lambda k: f"res{k}"))

        # ============== transpose back + store ==============
        for n in range(NT):
            onat = tmp.tile([P, C], MMDT, name="nat", bufs=2)
            for k in range(KC):
                pool = ypool if k % 2 == 0 else spool
                pt = pool.tile([P, 512], MMDT,
                               name="ypot" if k % 2 == 0 else "sps")
                nc.tensor.transpose(pt[:P, :P],
                                    xoT[k][:, n * P:(n + 1) * P],
                                    ident[:, :])
                if k % 2 == 0:
                    nc.scalar.copy(onat[:, k * P:(k + 1) * P], pt[:P, :P])
                else:
                    nc.vector.tensor_copy(onat[:, k * P:(k + 1) * P],
                                          pt[:P, :P])
            nc.sync.dma_start(out_d[n * P:(n + 1) * P, :], onat[:, :])

    nc.compile()
    return nc


# ---------------------------------------------------------------------------
# Entry point
# ---------------------------------------------------------------------------

_CACHE = {}


def _get_program(cfg: Cfg):
    if cfg not in _CACHE:
        _CACHE[cfg] = build_program(cfg)
    return _CACHE[cfg]


def run(inputs, cfg: Cfg = Cfg(), trace: bool = False):
    from concourse.bass_utils import run_bass_kernel_spmd
    nc = _get_program(cfg)
    in_maps = _host_inputs(cfg, inputs)
    res = run_bass_kernel_spmd(nc, in_maps, core_ids=list(range(cfg.NCORE)),
                               trace=trace)
    outs = [np.asarray(res.results[c]["out"], np.float32)
            for c in range(cfg.NCORE)]
    full = np.concatenate(outs, axis=0).reshape(cfg.B, cfg.T, cfg.C)
    return np.ascontiguousarray(full), res


def kernel(**inputs):
    out, _ = run(inputs)
    return out
